# revision 1
# baseline (speedup 1.0000x reference)
"""Trainium2 Bass kernel for nn_Attention (B=4, S=2048, D=1024, DK=256).

Computation (reference, per batch b):
    qp = q @ Wq.T            [S, DK]
    kp = q @ Wk.T            [S, DK]
    scores = qp @ kp.T / sqrt(DK)
    attn = softmax(scores, axis=-1)
    out = attn @ q           (v = q)
    y = out @ Wv.T           [S, D]

Sharding: 8 cores = 4 batches x 2 query-halves. Each core handles one batch's
full key/value sequence and one 1024-row query half. The host "rolls" the
sequence per core so that the core's query half occupies rows 0..1023; since
softmax is invariant to key permutation this changes nothing numerically.

Per-core dataflow (all matmul operands float32r: ~1.5e-4 rms error, 1 cyc/row):
    inputs: qn [S, D] (rolled, natural), qT [D, S] (rolled, transposed),
            wqT/wkT [D, DK], wvT [D, D]   (weights pre-transposed on host)
    kpT[e, s_k] = wkT.T @ qT              (PE, acc over 8 d-tiles)
    qpT[e, s_q] = wqT.T @ qT[:, :1024]
    per s_q chunk of 512:
      scoresT[s_k, s_q] = kpT.T @ qpT     (16 k-tiles x 2 e-acc)
      expT = exp(scoresT / 16)            (ScalarE, PSUM->SBUF, fused scale)
      denom[1, s_q] = ones.T @ expT       (acc over k-tiles)
        -> PE-transpose 128-blocks -> reciprocal -> recip[s_q part, 1]
      unnormT[d, s_q] = qn.T @ expT       (8 d-tiles x 16 k-acc, 2 groups of 4)
      y[s_q, e_out] = unnormT.T @ wvT     (8 d-acc)
      y *= recip (per-partition) -> DMA out
"""

import numpy as np

import concourse.mybir as mybir
import concourse.tile as tile
from concourse import bacc
from concourse.bass_utils import run_bass_kernel_spmd
from concourse.masks import make_identity

B, S, D, DK = 4, 2048, 1024, 256
SQ = S // 2  # query rows per core
P = 128
CH = 512  # s_q chunk width
N_CORES = 8

FR = mybir.dt.float32r
F32 = mybir.dt.float32

KT = S // P  # 16 key tiles
DT = D // P  # 8 d tiles
ET = DK // P  # 2 e tiles

_PROGRAM = None


def _build_program():
    nc = bacc.Bacc(None, target_bir_lowering=False, debug=False)

    qn_d = nc.dram_tensor("qn", [S, D], FR, kind="ExternalInput")
    qt_d = nc.dram_tensor("qt", [D, S], FR, kind="ExternalInput")
    wqt_d = nc.dram_tensor("wqt", [D, DK], FR, kind="ExternalInput")
    wkt_d = nc.dram_tensor("wkt", [D, DK], FR, kind="ExternalInput")
    wvt_d = nc.dram_tensor("wvt", [D, D], FR, kind="ExternalInput")
    y_d = nc.dram_tensor("y", [SQ, D], F32, kind="ExternalOutput")

    with tile.TileContext(nc) as tc:
        with tc.tile_pool(name="persist", bufs=1) as pp:
            # Constants
            ones_f = pp.tile([P, 1], F32, tag="ones_f")
            nc.vector.memset(ones_f[:], 1.0)
            ones = pp.tile([P, 1], FR, tag="ones")
            nc.vector.tensor_copy(ones[:], ones_f[:])
            ident = pp.tile([P, P], F32, tag="ident")
            make_identity(nc, ident[:])

            # Persistent SBUF arrays
            qn = [pp.tile([P, D], FR, tag="qn", bufs=KT, name=f"qn{k}") for k in range(KT)]
            kpt = [pp.tile([P, S], FR, tag="kpt", bufs=ET, name=f"kpt{e}") for e in range(ET)]
            qpt = [pp.tile([P, SQ], FR, tag="qpt", bufs=ET, name=f"qpt{e}") for e in range(ET)]

            for k in range(KT):
                nc.sync.dma_start(qn[k][:], qn_d[k * P : (k + 1) * P, :])

            # ---- Phase 1: projections ----
            with (
                tc.tile_pool(name="ph1", bufs=1) as p1,
                tc.tile_pool(name="ps1", bufs=1, space="PSUM") as ps1,
            ):
                qt = [p1.tile([P, S], FR, tag="qt", bufs=DT, name=f"qt{d}") for d in range(DT)]
                wqt = [p1.tile([P, DK], FR, tag="wqt", bufs=DT, name=f"wqt{d}") for d in range(DT)]
                wkt = [p1.tile([P, DK], FR, tag="wkt", bufs=DT, name=f"wkt{d}") for d in range(DT)]
                for d in range(DT):
                    nc.sync.dma_start(qt[d][:], qt_d[d * P : (d + 1) * P, :])
                    nc.sync.dma_start(wqt[d][:], wqt_d[d * P : (d + 1) * P, :])
                    nc.sync.dma_start(wkt[d][:], wkt_d[d * P : (d + 1) * P, :])

                # kpT[e] spans all S keys; qpT[e] spans our SQ queries (qt cols 0..SQ)
                for e in range(ET):
                    for n in range(S // 512):
                        pk = ps1.tile([P, 512], F32, tag="proj", bufs=4, name=f"pk{e}_{n}")
                        for d in range(DT):
                            nc.tensor.matmul(
                                pk[:],
                                wkt[d][:, e * P : (e + 1) * P],
                                qt[d][:, n * 512 : (n + 1) * 512],
                                start=(d == 0),
                                stop=(d == DT - 1),
                            )
                        nc.vector.tensor_copy(kpt[e][:, n * 512 : (n + 1) * 512], pk[:])
                    for n in range(SQ // 512):
                        pq = ps1.tile([P, 512], F32, tag="proj", bufs=4, name=f"pq{e}_{n}")
                        for d in range(DT):
                            nc.tensor.matmul(
                                pq[:],
                                wqt[d][:, e * P : (e + 1) * P],
                                qt[d][:, n * 512 : (n + 1) * 512],
                                start=(d == 0),
                                stop=(d == DT - 1),
                            )
                        nc.vector.tensor_copy(qpt[e][:, n * 512 : (n + 1) * 512], pq[:])

            # ---- Phase 2: attention, in s_q chunks of CH ----
            with (
                tc.tile_pool(name="ph2", bufs=1) as p2,
                tc.tile_pool(name="ps2", bufs=1, space="PSUM") as ps2,
            ):
                wvt = [p2.tile([P, D], FR, tag="wvt", bufs=DT, name=f"wvt{d}") for d in range(DT)]
                for d in range(DT):
                    nc.sync.dma_start(wvt[d][:], wvt_d[d * P : (d + 1) * P, :])

                for c in range(SQ // CH):
                    cs = c * CH
                    # scores^T + exp, per key tile
                    expt = []
                    for k in range(KT):
                        sc = ps2.tile([P, CH], F32, tag="sc", bufs=2, name=f"sc{c}_{k}")
                        for e in range(ET):
                            nc.tensor.matmul(
                                sc[:],
                                kpt[e][:, k * P : (k + 1) * P],
                                qpt[e][:, cs : cs + CH],
                                start=(e == 0),
                                stop=(e == ET - 1),
                            )
                        ex = p2.tile([P, CH], FR, tag="expt", bufs=18, name=f"ex{c}_{k}")
                        nc.scalar.activation(
                            ex[:], sc[:], mybir.ActivationFunctionType.Exp, scale=1.0 / 16.0
                        )
                        expt.append(ex)

                    # denominator: ones^T @ expT, accumulated over key tiles
                    pd = ps2.tile([1, CH], F32, tag="pd", bufs=2, name=f"pd{c}")
                    for k in range(KT):
                        nc.tensor.matmul(
                            pd[:], ones[:], expt[k][:], start=(k == 0), stop=(k == KT - 1)
                        )
                    drow = p2.tile([1, CH], F32, tag="drow", bufs=2, name=f"drow{c}")
                    nc.vector.tensor_copy(drow[:], pd[:])
                    pt = ps2.tile([P, CH // P], F32, tag="pd", bufs=2, name=f"pt{c}")
                    for j in range(CH // P):
                        nc.tensor.transpose(
                            pt[:, j : j + 1], drow[:1, j * P : (j + 1) * P], ident[:1, :1]
                        )
                    recip = p2.tile([P, CH // P], F32, tag="recip", bufs=2, name=f"recip{c}")
                    nc.vector.reciprocal(recip[:], pt[:])

                    # unnorm^T[d, s_q] = qn.T @ expT, d-groups of 4 banks
                    unsb = []
                    for g in range(2):
                        accs = [
                            ps2.tile([P, CH], F32, tag="un", bufs=4, name=f"un{c}_{g}_{i}")
                            for i in range(4)
                        ]
                        for k in range(KT):
                            for i in range(4):
                                d = g * 4 + i
                                nc.tensor.matmul(
                                    accs[i][:],
                                    qn[k][:, d * P : (d + 1) * P],
                                    expt[k][:],
                                    start=(k == 0),
                                    stop=(k == KT - 1),
                                )
                        for i in range(4):
                            us = p2.tile([P, CH], FR, tag="unsb", bufs=8, name=f"us{c}_{g}_{i}")
                            nc.vector.tensor_copy(us[:], accs[i][:])
                            unsb.append(us)

                    # y[s_q, e_out] = unnorm^T.T @ wvT, then normalize rows
                    for m in range(CH // P):
                        for n in range(D // 512):
                            yb = ps2.tile([P, 512], F32, tag="sc", bufs=2, name=f"yb{c}_{m}_{n}")
                            for d in range(DT):
                                nc.tensor.matmul(
                                    yb[:],
                                    unsb[d][:, m * P : (m + 1) * P],
                                    wvt[d][:, n * 512 : (n + 1) * 512],
                                    start=(d == 0),
                                    stop=(d == DT - 1),
                                )
                            ys = p2.tile([P, 512], F32, tag="ysb", bufs=4, name=f"ys{c}_{m}_{n}")
                            nc.vector.tensor_scalar_mul(ys[:], yb[:], recip[:, m : m + 1])
                            nc.sync.dma_start(
                                y_d[cs + m * P : cs + (m + 1) * P, n * 512 : (n + 1) * 512],
                                ys[:],
                            )

    nc.compile()
    return nc


def kernel(q, Wq, Wk, Wv):
    global _PROGRAM
    if _PROGRAM is None:
        _PROGRAM = _build_program()
    nc = _PROGRAM

    q = np.ascontiguousarray(np.asarray(q, dtype=np.float32))
    wqt = np.ascontiguousarray(np.asarray(Wq, dtype=np.float32).T)
    wkt = np.ascontiguousarray(np.asarray(Wk, dtype=np.float32).T)
    wvt = np.ascontiguousarray(np.asarray(Wv, dtype=np.float32).T)

    in_maps = []
    for core in range(N_CORES):
        b, h = divmod(core, 2)
        qb = q[b]
        rolled = np.concatenate([qb[h * SQ : (h + 1) * SQ], qb[(1 - h) * SQ : (2 - h) * SQ]])
        in_maps.append(
            {
                "qn": np.ascontiguousarray(rolled),
                "qt": np.ascontiguousarray(rolled.T),
                "wqt": wqt,
                "wkt": wkt,
                "wvt": wvt,
            }
        )

    res = run_bass_kernel_spmd(nc, in_maps, list(range(N_CORES)))

    out = np.empty((B, S, D), dtype=np.float32)
    for core in range(N_CORES):
        b, h = divmod(core, 2)
        out[b, h * SQ : (h + 1) * SQ, :] = res.results[core]["y"]
    return out


# revision 3
# speedup vs baseline: 1.2416x; 1.2416x over previous
"""Trainium2 Bass kernel for nn_Attention (B=4, S=2048, D=1024, DK=256).

Computation (reference, per batch b):
    qp = q @ Wq.T            [S, DK]
    kp = q @ Wk.T            [S, DK]
    scores = qp @ kp.T / sqrt(DK)
    attn = softmax(scores, axis=-1)
    out = attn @ q           (v = q)
    y = out @ Wv.T           [S, D]

Sharding: 8 cores = 4 batches x 2 query-halves. Each core handles one batch's
full key/value sequence and one 1024-row query half. The host "rolls" the
sequence per core so that the core's query half occupies rows 0..1023; since
softmax is invariant to key permutation this changes nothing numerically.

Per-core dataflow (all matmul operands float32r: ~1.5e-4 rms error, 1 cyc/row):
    inputs: qn [S, D] (rolled, natural), qT [D, S] (rolled, transposed),
            wqT/wkT [D, DK], wvT [D, D]   (weights pre-transposed on host)
    kpT[e, s_k] = wkT.T @ qT              (PE, acc over 8 d-tiles)
    qpT[e, s_q] = wqT.T @ qT[:, :1024]
    per s_q chunk of 512:
      scoresT[s_k, s_q] = kpT.T @ qpT     (16 k-tiles x 2 e-acc)
      expT = exp(scoresT / 16)            (ScalarE, PSUM->SBUF, fused scale)
      denom[1, s_q] = ones.T @ expT       (acc over k-tiles)
        -> PE-transpose 128-blocks -> reciprocal -> recip[s_q part, 1]
      unnormT[d, s_q] = qn.T @ expT       (8 d-tiles x 16 k-acc, 2 groups of 4)
      y[s_q, e_out] = unnormT.T @ wvT     (8 d-acc)
      y *= recip (per-partition) -> DMA out
"""

import numpy as np

import concourse.mybir as mybir
import concourse.tile as tile
from concourse import bacc
from concourse.bass_utils import run_bass_kernel_spmd
from concourse.masks import make_identity

B, S, D, DK = 4, 2048, 1024, 256
SQ = S // 2  # query rows per core
P = 128
CH = 512  # s_q chunk width
N_CORES = 8

FR = mybir.dt.float32r
F32 = mybir.dt.float32

KT = S // P  # 16 key tiles
DT = D // P  # 8 d tiles
ET = DK // P  # 2 e tiles

_PROGRAM = None


def _build_program():
    nc = bacc.Bacc(None, target_bir_lowering=False, debug=False)

    qn_d = nc.dram_tensor("qn", [S, D], FR, kind="ExternalInput")
    qt_d = nc.dram_tensor("qt", [D, S], FR, kind="ExternalInput")
    wqt_d = nc.dram_tensor("wqt", [D, DK], FR, kind="ExternalInput")
    wkt_d = nc.dram_tensor("wkt", [D, DK], FR, kind="ExternalInput")
    wvt_d = nc.dram_tensor("wvt", [D, D], FR, kind="ExternalInput")
    y_d = nc.dram_tensor("y", [SQ, D], F32, kind="ExternalOutput")

    with tile.TileContext(nc) as tc:
        with tc.tile_pool(name="persist", bufs=1) as pp:
            # Constants
            ones_f = pp.tile([P, 1], F32, tag="ones_f")
            nc.vector.memset(ones_f[:], 1.0)
            ones = pp.tile([P, 1], FR, tag="ones")
            nc.vector.tensor_copy(ones[:], ones_f[:])
            ident = pp.tile([P, P], F32, tag="ident")
            make_identity(nc, ident[:])

            # Persistent SBUF arrays
            qn = [pp.tile([P, D], FR, tag="qn", bufs=KT, name=f"qn{k}") for k in range(KT)]
            kpt = [pp.tile([P, S], FR, tag="kpt", bufs=ET, name=f"kpt{e}") for e in range(ET)]
            qpt = [pp.tile([P, SQ], FR, tag="qpt", bufs=ET, name=f"qpt{e}") for e in range(ET)]

            # ---- Phase 1: projections ----
            with (
                tc.tile_pool(name="ph1", bufs=1) as p1,
                tc.tile_pool(name="ps1", bufs=1, space="PSUM") as ps1,
            ):
                qt = [p1.tile([P, S], FR, tag="qt", bufs=DT, name=f"qt{d}") for d in range(DT)]
                wqt = [p1.tile([P, DK], FR, tag="wqt", bufs=DT, name=f"wqt{d}") for d in range(DT)]
                wkt = [p1.tile([P, DK], FR, tag="wkt", bufs=DT, name=f"wkt{d}") for d in range(DT)]
                # DMA priority: weights + qt tiles first (projections need them
                # immediately, in ascending-d order); qn next (needed at unnorm);
                # wvt is loaded in phase 2 (needed only at the final matmul).
                for d in range(DT):
                    nc.sync.dma_start(wkt[d][:], wkt_d[d * P : (d + 1) * P, :])
                    nc.sync.dma_start(wqt[d][:], wqt_d[d * P : (d + 1) * P, :])
                    nc.sync.dma_start(qt[d][:], qt_d[d * P : (d + 1) * P, :])
                for k in range(KT):
                    nc.sync.dma_start(qn[k][:], qn_d[k * P : (k + 1) * P, :])

                # d-outer accumulation so compute starts as soon as qt[0] lands.
                # kp first (8 psum banks), then qp (4 banks, same tag slots).
                pk = {
                    (e, n): ps1.tile([P, 512], F32, tag="proj", bufs=8, name=f"pk{e}_{n}")
                    for e in range(ET)
                    for n in range(S // 512)
                }
                for d in range(DT):
                    for e in range(ET):
                        for n in range(S // 512):
                            nc.tensor.matmul(
                                pk[e, n][:],
                                wkt[d][:, e * P : (e + 1) * P],
                                qt[d][:, n * 512 : (n + 1) * 512],
                                start=(d == 0),
                                stop=(d == DT - 1),
                            )
                for e in range(ET):
                    for n in range(S // 512):
                        nc.vector.tensor_copy(kpt[e][:, n * 512 : (n + 1) * 512], pk[e, n][:])
                pq = {
                    (e, n): ps1.tile([P, 512], F32, tag="proj", bufs=8, name=f"pq{e}_{n}")
                    for e in range(ET)
                    for n in range(SQ // 512)
                }
                for d in range(DT):
                    for e in range(ET):
                        for n in range(SQ // 512):
                            nc.tensor.matmul(
                                pq[e, n][:],
                                wqt[d][:, e * P : (e + 1) * P],
                                qt[d][:, n * 512 : (n + 1) * 512],
                                start=(d == 0),
                                stop=(d == DT - 1),
                            )
                for e in range(ET):
                    for n in range(SQ // 512):
                        nc.vector.tensor_copy(qpt[e][:, n * 512 : (n + 1) * 512], pq[e, n][:])

            # ---- Phase 2: attention, in s_q chunks of CH ----
            with (
                tc.tile_pool(name="ph2", bufs=1) as p2,
                tc.tile_pool(name="ps2", bufs=1, space="PSUM") as ps2,
            ):
                wvt = [p2.tile([P, D], FR, tag="wvt", bufs=DT, name=f"wvt{d}") for d in range(DT)]
                for d in range(DT):
                    nc.sync.dma_start(wvt[d][:], wvt_d[d * P : (d + 1) * P, :])

                for c in range(SQ // CH):
                    cs = c * CH
                    # scores^T + exp, per key tile
                    expt = []
                    for k in range(KT):
                        sc = ps2.tile([P, CH], F32, tag="sc", bufs=2, name=f"sc{c}_{k}")
                        for e in range(ET):
                            nc.tensor.matmul(
                                sc[:],
                                kpt[e][:, k * P : (k + 1) * P],
                                qpt[e][:, cs : cs + CH],
                                start=(e == 0),
                                stop=(e == ET - 1),
                            )
                        ex = p2.tile([P, CH], FR, tag="expt", bufs=18, name=f"ex{c}_{k}")
                        nc.scalar.activation(
                            ex[:], sc[:], mybir.ActivationFunctionType.Exp, scale=1.0 / 16.0
                        )
                        expt.append(ex)

                    # denominator: DVE-accumulate exp tiles (keeps PE free), then
                    # one ones-matmul for the partition sum, tiny PE transposes,
                    # reciprocal.
                    dacc = p2.tile([P, CH], F32, tag="dacc", bufs=2, name=f"dacc{c}")
                    nc.vector.tensor_copy(dacc[:], expt[0][:])
                    for k in range(1, KT):
                        nc.vector.tensor_tensor(
                            dacc[:], dacc[:], expt[k][:], op=mybir.AluOpType.add
                        )
                    daccr = p2.tile([P, CH], FR, tag="daccr", bufs=2, name=f"daccr{c}")
                    nc.vector.tensor_copy(daccr[:], dacc[:])
                    pd = ps2.tile([1, CH], F32, tag="pd", bufs=1, name=f"pd{c}")
                    nc.tensor.matmul(pd[:], ones[:], daccr[:], start=True, stop=True)
                    drow = p2.tile([1, CH], F32, tag="drow", bufs=2, name=f"drow{c}")
                    nc.vector.tensor_copy(drow[:], pd[:])
                    pt = ps2.tile([P, CH // P], F32, tag="pd", bufs=1, name=f"pt{c}")
                    for j in range(CH // P):
                        nc.tensor.transpose(
                            pt[:, j : j + 1], drow[:1, j * P : (j + 1) * P], ident[:1, :1]
                        )
                    recip = p2.tile([P, CH // P], F32, tag="recip", bufs=2, name=f"recip{c}")
                    nc.vector.reciprocal(recip[:], pt[:])

                    # unnorm^T[d, s_q] = qn.T @ expT, d-groups of 4 banks
                    unsb = []
                    for g in range(2):
                        accs = [
                            ps2.tile([P, CH], F32, tag="un", bufs=4, name=f"un{c}_{g}_{i}")
                            for i in range(4)
                        ]
                        for k in range(KT):
                            for i in range(4):
                                d = g * 4 + i
                                nc.tensor.matmul(
                                    accs[i][:],
                                    qn[k][:, d * P : (d + 1) * P],
                                    expt[k][:],
                                    start=(k == 0),
                                    stop=(k == KT - 1),
                                )
                        for i in range(4):
                            us = p2.tile([P, CH], FR, tag="unsb", bufs=8, name=f"us{c}_{g}_{i}")
                            nc.vector.tensor_copy(us[:], accs[i][:])
                            unsb.append(us)

                    # y[s_q, e_out] = unnorm^T.T @ wvT, then normalize rows
                    for m in range(CH // P):
                        for n in range(D // 512):
                            yb = ps2.tile([P, 512], F32, tag="sc", bufs=2, name=f"yb{c}_{m}_{n}")
                            for d in range(DT):
                                nc.tensor.matmul(
                                    yb[:],
                                    unsb[d][:, m * P : (m + 1) * P],
                                    wvt[d][:, n * 512 : (n + 1) * 512],
                                    start=(d == 0),
                                    stop=(d == DT - 1),
                                )
                            ys = p2.tile([P, 512], F32, tag="ysb", bufs=4, name=f"ys{c}_{m}_{n}")
                            nc.vector.tensor_scalar_mul(ys[:], yb[:], recip[:, m : m + 1])
                            nc.sync.dma_start(
                                y_d[cs + m * P : cs + (m + 1) * P, n * 512 : (n + 1) * 512],
                                ys[:],
                            )

    nc.compile()
    return nc


def kernel(q, Wq, Wk, Wv):
    global _PROGRAM
    if _PROGRAM is None:
        _PROGRAM = _build_program()
    nc = _PROGRAM

    q = np.ascontiguousarray(np.asarray(q, dtype=np.float32))
    wqt = np.ascontiguousarray(np.asarray(Wq, dtype=np.float32).T)
    wkt = np.ascontiguousarray(np.asarray(Wk, dtype=np.float32).T)
    wvt = np.ascontiguousarray(np.asarray(Wv, dtype=np.float32).T)

    in_maps = []
    for core in range(N_CORES):
        b, h = divmod(core, 2)
        qb = q[b]
        rolled = np.concatenate([qb[h * SQ : (h + 1) * SQ], qb[(1 - h) * SQ : (2 - h) * SQ]])
        in_maps.append(
            {
                "qn": np.ascontiguousarray(rolled),
                "qt": np.ascontiguousarray(rolled.T),
                "wqt": wqt,
                "wkt": wkt,
                "wvt": wvt,
            }
        )

    res = run_bass_kernel_spmd(nc, in_maps, list(range(N_CORES)))

    out = np.empty((B, S, D), dtype=np.float32)
    for core in range(N_CORES):
        b, h = divmod(core, 2)
        out[b, h * SQ : (h + 1) * SQ, :] = res.results[core]["y"]
    return out
